# revision 4
# baseline (speedup 1.0000x reference)
"""Trainium2 Bass kernel for nn_Attention_49452253446746.

Full fused attention: qkv projection + interleaved RoPE (with pe_scale) +
masked softmax + attention output, for x(4,2048,1024), 16 heads of d=64.

Sharding: 8 cores = 4 batches x 2 head-groups (8 heads each). Each core
computes out[b, :, g*512:(g+1)*512].

Key layout choices (per core):
- Projection computes q/k directly in (d, t) "transposed" layout via
  lhsT=W^T tiles, rhs=x^T tiles; v in natural (t, d) layout.
- RoPE is applied as q' = (q+bq) * A + (qrot+bqrot) * B where qrot is a
  second projection with host-rotated weight rows, and A/B are host-built
  (d,t) tables folding cos/sin, pe_scale and the attention scale.
- scores^T[k,q] = k'^T q' (contraction d=64, row-tiled 2 heads per 128
  partitions), psum-accumulated; softmax denominators come from a 65th
  "ones" column appended to v in the p@v matmul; mask is folded as
  elementwise exp(mask) multiplied into exp(scores) (bf16, DVE 2x mode).
- Final (d,q)->(q,d) transpose on the tensor engine, divide by sums via
  per-partition reciprocal, DMA straight out.
"""

import sys

for _p in ("/opt/trn_rl_repo",):
    if _p not in sys.path:
        sys.path.insert(0, _p)

import numpy as np
import ml_dtypes

import concourse.bacc as bacc
import concourse.mybir as mybir
from concourse.tile import TileContext
from concourse.masks import make_identity
from concourse.bass_utils import run_bass_kernel_spmd

BF16 = mybir.dt.bfloat16
F32 = mybir.dt.float32
AF = mybir.ActivationFunctionType
ALU = mybir.AluOpType

B, T_FULL, C, NH, D = 4, 2048, 1024, 16, 64
N_CORES = 8
HPC = 8          # heads per core
PAIRS = 4        # head pairs per core
CO = C // 128    # 8 contraction tiles for the projection
JW = 2560        # w columns: q(512) qrot(512) k(512) krot(512) v(512)
SCALE = 1.0 / float(np.sqrt(2.0 * D))


def _build(T=T_FULL):
    TC_ = T // 512   # 512-wide q/t chunks
    TT = T // 128    # 128-wide t tiles
    KT = T // 128    # 128-wide k tiles
    KG = KT // 2     # k-tile groups of 2 (exp reads 2 psum banks at once)

    nc = bacc.Bacc("TRN2", target_bir_lowering=False, debug=False,
                   num_devices=N_CORES)
    xT = nc.dram_tensor("xT", (C, T), BF16, kind="ExternalInput")
    wT = nc.dram_tensor("wT", (C, JW), BF16, kind="ExternalInput")
    ab = nc.dram_tensor("ab", (128, 4, T), BF16, kind="ExternalInput")
    bias = nc.dram_tensor("bias", (128, 16), F32, kind="ExternalInput")
    bv = nc.dram_tensor("bv", (1, 512), BF16, kind="ExternalInput")
    em = nc.dram_tensor("em", (T, T), BF16, kind="ExternalInput")
    out = nc.dram_tensor("out", (T, 512), F32, kind="ExternalOutput")

    with TileContext(nc) as tc:
        with (
            tc.tile_pool(name="const", bufs=1) as constp,
            tc.tile_pool(name="work", bufs=2) as workp,
            tc.tile_pool(name="eb", bufs=3) as ebp,
            tc.tile_pool(name="fin", bufs=2) as finp,
            tc.tile_pool(name="pp", bufs=2, space="PSUM") as proj_ps,
            tc.tile_pool(name="sp", bufs=2, space="PSUM") as score_ps,
            tc.tile_pool(name="vp", bufs=1, space="PSUM") as pv_ps,
            tc.tile_pool(name="tp", bufs=1, space="PSUM") as tr_ps,
        ):
            x_sb = constp.tile([128, CO, T], BF16)
            nc.sync.dma_start(x_sb, xT.ap().rearrange("(co p) t -> p co t", p=128))
            w_sb = constp.tile([128, CO, JW], BF16)
            nc.sync.dma_start(w_sb, wT.ap().rearrange("(co p) j -> p co j", p=128))
            ab_sb = constp.tile([128, 4, T], BF16)
            nc.sync.dma_start(ab_sb, ab.ap())
            bias_sb = constp.tile([128, 16], F32)
            nc.sync.dma_start(bias_sb, bias.ap())
            bv_sb = constp.tile([1, 512], BF16)
            nc.sync.dma_start(bv_sb, bv.ap())
            ident = constp.tile([128, 128], BF16)
            make_identity(nc, ident)
            ones1 = constp.tile([1, 128], BF16)
            nc.vector.memset(ones1, 1.0)

            # v with a 65th all-ones column per head (softmax denominators)
            v_sb = constp.tile([128, KT, HPC * 65], BF16)
            v_sb_h = v_sb.rearrange("p k (h y) -> p k h y", y=65)
            nc.vector.memset(v_sb_h[:, :, :, 64], 1.0)

            # q'/k' in (d, t) layout; j-tiles: q pairs 0-3, k pairs 4-7
            qk_sb = constp.tile([128, 8, T], BF16)

            # ---- projection + rope ----
            for p_ in range(PAIRS):
                for tcx in range(TC_):
                    tsl = slice(tcx * 512, (tcx + 1) * 512)
                    for which, joff, aoff, dst in (
                        ("q", p_, 0, p_),
                        ("k", 8 + p_, 2, 4 + p_),
                    ):
                        ps_m = proj_ps.tile([128, 512], F32, tag="pp")
                        for co in range(CO):
                            nc.tensor.matmul(
                                ps_m,
                                w_sb[:, co, joff * 128:(joff + 1) * 128],
                                x_sb[:, co, tsl],
                                start=(co == 0), stop=(co == CO - 1))
                        ps_r = proj_ps.tile([128, 512], F32, tag="pp")
                        roff = joff + 4
                        for co in range(CO):
                            nc.tensor.matmul(
                                ps_r,
                                w_sb[:, co, roff * 128:(roff + 1) * 128],
                                x_sb[:, co, tsl],
                                start=(co == 0), stop=(co == CO - 1))
                        s1 = workp.tile([128, 512], F32, tag="s1")
                        nc.vector.scalar_tensor_tensor(
                            s1, ps_m, bias_sb[:, joff:joff + 1],
                            ab_sb[:, aoff, tsl], ALU.add, ALU.mult)
                        s2 = workp.tile([128, 512], F32, tag="s2")
                        nc.vector.scalar_tensor_tensor(
                            s2, ps_r, bias_sb[:, roff:roff + 1],
                            ab_sb[:, aoff + 1, tsl], ALU.add, ALU.mult)
                        nc.vector.tensor_add(qk_sb[:, dst, tsl], s1, s2)

            # ---- v projection ----
            for tt in range(TT):
                psv = proj_ps.tile([128, 512], F32, tag="pp")
                for co in range(CO):
                    nc.tensor.matmul(
                        psv,
                        x_sb[:, co, tt * 128:(tt + 1) * 128],
                        w_sb[:, co, 2048:2560],
                        start=(co == 0), stop=False)
                # bias via K=1 matmul: psv[t, j] += 1 * bv[j]
                nc.tensor.matmul(psv, ones1, bv_sb, start=False, stop=True)
                nc.vector.tensor_copy(
                    v_sb_h[:, tt, :, :64],
                    psv.rearrange("p (h d) -> p h d", d=64))

            # ---- attention ----
            for qcx in range(TC_):
                qsl = slice(qcx * 512, (qcx + 1) * 512)
                em_t = workp.tile([128, KT, 512], BF16, tag="em")
                nc.sync.dma_start(
                    em_t, em.ap()[:, qsl].rearrange("(kt p) q -> p kt q", p=128))
                for p_ in range(PAIRS):
                    for hh in range(2):
                        h = 2 * p_ + hh
                        pb = hh * 64
                        pv = pv_ps.tile([65, 512], F32, tag="pv")
                        for kg in range(KG):
                            sc = score_ps.tile([128, 2, 512], F32, tag="sc")
                            for j2 in range(2):
                                kt = kg * 2 + j2
                                nc.tensor.matmul(
                                    sc[:, j2],
                                    qk_sb[pb:pb + 64, 4 + p_,
                                          kt * 128:(kt + 1) * 128],
                                    qk_sb[pb:pb + 64, p_, qsl],
                                    start=True, stop=True,
                                    tile_position=(pb, 0))
                            e_t = ebp.tile([128, 2, 512], BF16, tag="e")
                            nc.scalar.activation(e_t, sc, AF.Exp)
                            ep_t = ebp.tile([128, 2, 512], BF16, tag="ep")
                            nc.vector.tensor_mul(
                                ep_t, e_t, em_t[:, kg * 2:kg * 2 + 2])
                            for j2 in range(2):
                                nc.tensor.matmul(
                                    pv,
                                    v_sb_h[:, kg * 2 + j2, h, :],
                                    ep_t[:, j2],
                                    start=(kg == 0 and j2 == 0),
                                    stop=(kg == KG - 1 and j2 == 1))
                        # finalize: transpose (65,512) -> 4x (128,65), divide
                        o_sb = finp.tile([65, 512], BF16, tag="osb")
                        nc.vector.tensor_copy(o_sb, pv)
                        tr = tr_ps.tile([128, 4, 66], BF16, tag="tr")
                        for b4 in range(4):
                            nc.tensor.transpose(
                                tr[:, b4, :65],
                                o_sb[:, b4 * 128:(b4 + 1) * 128],
                                ident[:65, :65])
                        ob = finp.tile([128, 4, 64], F32, tag="ob")
                        for b4 in range(4):
                            rc = finp.tile([128, 1], F32, tag="rc")
                            nc.vector.reciprocal(rc, tr[:, b4, 64:65])
                            nc.vector.tensor_scalar_mul(
                                ob[:, b4], tr[:, b4, :64], rc)
                        nc.sync.dma_start(
                            out.ap()[qsl, h * 64:(h + 1) * 64].rearrange(
                                "(b q) d -> q b d", q=128),
                            ob)
    nc.compile()
    return nc


def _rot_rows(wm):
    """RoPE interleave on the leading axis: row 2i -> -row(2i+1), 2i+1 -> row 2i."""
    wr = wm.reshape((-1, 2) + wm.shape[1:])
    return np.stack([-wr[:, 1], wr[:, 0]], axis=1).reshape(wm.shape)


def _host_prep(inputs, T=T_FULL):
    bf = ml_dtypes.bfloat16
    x = np.asarray(inputs["x"], np.float32)
    pe_cos = np.asarray(inputs["pe_cos"], np.float32)[0, 0]      # (T, D)
    pe_sin = np.asarray(inputs["pe_sin"], np.float32)[0, 0]
    pe_scale = np.asarray(inputs["pe_scale"], np.float32)[0, 0]
    mask = np.asarray(inputs["mask"], np.float32)[0]             # (B, T, T)
    w = np.asarray(inputs["w_qkv"], np.float32)                  # (3C, C)
    b = np.asarray(inputs["b_qkv"], np.float32)

    cosT, sinT, scT = pe_cos.T, pe_sin.T, pe_scale.T             # (D, T)
    ab_host = np.stack([
        np.tile(cosT * scT * SCALE, (2, 1)),
        np.tile(sinT * scT * SCALE, (2, 1)),
        np.tile(cosT / scT, (2, 1)),
        np.tile(sinT / scT, (2, 1)),
    ], axis=1).astype(bf)                                        # (128, 4, T)

    in_maps = []
    for c in range(N_CORES):
        bidx, g = divmod(c, 2)
        gs = slice(g * 512, (g + 1) * 512)
        wq, wk, wv = w[:C][gs], w[C:2 * C][gs], w[2 * C:][gs]
        bq, bk, bv_ = b[:C][gs], b[C:2 * C][gs], b[2 * C:][gs]
        wqr, wkr = _rot_rows(wq), _rot_rows(wk)
        bqr, bkr = _rot_rows(bq), _rot_rows(bk)
        wT_host = np.ascontiguousarray(
            np.concatenate([wq, wqr, wk, wkr, wv], 0).T).astype(bf)
        bias16 = np.zeros((128, 16), np.float32)
        for p_ in range(PAIRS):
            ps = slice(p_ * 128, (p_ + 1) * 128)
            bias16[:, p_] = bq[ps]
            bias16[:, 4 + p_] = bqr[ps]
            bias16[:, 8 + p_] = bk[ps]
            bias16[:, 12 + p_] = bkr[ps]
        in_maps.append({
            "xT": np.ascontiguousarray(x[bidx].T).astype(bf),
            "wT": wT_host,
            "ab": ab_host,
            "bias": bias16,
            "bv": bv_.astype(bf)[None, :],
            "em": np.exp(mask[bidx].T).astype(bf),
        })
    return in_maps


def kernel(**inputs):
    in_maps = _host_prep(inputs)
    nc = _build()
    res = run_bass_kernel_spmd(nc, in_maps, core_ids=list(range(N_CORES)))
    full = np.empty((B, T_FULL, C), np.float32)
    for c in range(N_CORES):
        bidx, g = divmod(c, 2)
        full[bidx, :, g * 512:(g + 1) * 512] = res.results[c]["out"]
    return full


# revision 14
# speedup vs baseline: 64.3446x; 64.3446x over previous
"""Trainium2 Bass kernel for nn_Attention_49452253446746.

Full fused attention: qkv projection + interleaved RoPE (with pe_scale) +
masked softmax + attention output, for x(4,2048,1024), 16 heads of d=64.

Sharding: 8 cores = 4 batches x 2 head-groups (8 heads each). Each core
computes out[b, :, g*512:(g+1)*512].

Key layout choices (per core):
- Projection computes q/k directly in (d, t) "transposed" layout via
  lhsT=W^T tiles, rhs=x^T tiles; v in natural (t, d) layout.
- RoPE: q' = (q+bq)*A + (rot(q)+rot(bq))*B. rot() swaps adjacent partition
  pairs with a sign flip; the swap is a DVE stream_shuffle of the psum
  tile and the signs are folded into the host-built B tables, so no extra
  matmuls are needed. A/B also fold cos/sin, pe_scale and the attn scale.
- scores^T[k,q] = k'^T q' (contraction d=64). The two heads of a pair are
  row-tiled at array rows 0-63/64-127 and write the two banks of one
  (128,2,512) psum tile, so they run concurrently and one exp covers both.
- Softmax denominators come from a 65th "ones" column appended to v in the
  p@v matmul; mask is folded as exp(mask) multiplied into exp(scores)
  (bf16; split between DVE and GPSIMD).
- Final (d,q)->(q,d) transpose on the tensor engine, divide by sums via
  per-partition reciprocal, DMA straight out.
"""

import sys

for _p in ("/opt/trn_rl_repo",):
    if _p not in sys.path:
        sys.path.insert(0, _p)

import numpy as np
import ml_dtypes

import concourse.bacc as bacc
import concourse.mybir as mybir
from concourse.tile import TileContext
from concourse.masks import make_identity
from concourse.bass_utils import run_bass_kernel_spmd

BF16 = mybir.dt.bfloat16
F32 = mybir.dt.float32
AF = mybir.ActivationFunctionType
ALU = mybir.AluOpType

B, T_FULL, C, NH, D = 4, 2048, 1024, 16, 64
N_CORES = 8
HPC = 8          # heads per core
PAIRS = 4        # head pairs per core
CO = C // 128    # 8 contraction tiles for the projection
JW = 1536        # w columns: q(512) k(512) v(512)
SCALE = 1.0 / float(np.sqrt(2.0 * D))

# stream_shuffle mask: swap adjacent partition pairs within each 32-group
SWAP_MASK = [i ^ 1 for i in range(32)]


def _build(T=T_FULL):
    TC_ = T // 512   # 512-wide q/t chunks
    TT = T // 128    # 128-wide t tiles
    KT = T // 128    # 128-wide k tiles

    nc = bacc.Bacc("TRN2", target_bir_lowering=False, debug=False,
                   num_devices=N_CORES)
    xT = nc.dram_tensor("xT", (C, T), BF16, kind="ExternalInput")
    wT = nc.dram_tensor("wT", (C, JW), BF16, kind="ExternalInput")
    ab = nc.dram_tensor("ab", (128, 4, T), BF16, kind="ExternalInput")
    bias = nc.dram_tensor("bias", (128, 16), F32, kind="ExternalInput")
    bv = nc.dram_tensor("bv", (1, 512), BF16, kind="ExternalInput")
    em = nc.dram_tensor("em", (T, T), BF16, kind="ExternalInput")
    out = nc.dram_tensor("out", (T, 512), F32, kind="ExternalOutput")

    with TileContext(nc) as tc:
        with (
            tc.tile_pool(name="const", bufs=1) as constp,
            tc.tile_pool(name="work", bufs=2) as workp,
            tc.tile_pool(name="eb", bufs=4) as ebp,
            tc.tile_pool(name="fin", bufs=4) as finp,
            tc.tile_pool(name="pp", bufs=2, space="PSUM") as proj_ps,
            tc.tile_pool(name="sp", bufs=2, space="PSUM") as score_ps,
            tc.tile_pool(name="vp", bufs=1, space="PSUM") as pv_ps,
        ):
            x_sb = constp.tile([128, CO, T], BF16)
            xT_r = xT.ap().rearrange("(co p) t -> p co t", p=128)
            w_sb = constp.tile([128, CO, JW], BF16)
            wT_r = wT.ap().rearrange("(co p) j -> p co j", p=128)
            for co in range(CO):
                nc.sync.dma_start(w_sb[:, co], wT_r[:, co])
                nc.sync.dma_start(x_sb[:, co], xT_r[:, co])
            ab_sb = constp.tile([128, 4, T], BF16)
            nc.sync.dma_start(ab_sb, ab.ap())
            bias_sb = constp.tile([128, 16], F32)
            nc.sync.dma_start(bias_sb, bias.ap())
            bv_sb = constp.tile([1, 512], BF16)
            nc.sync.dma_start(bv_sb, bv.ap())
            ident = constp.tile([128, 128], BF16)
            make_identity(nc, ident)
            ones1 = constp.tile([1, 128], BF16)
            nc.vector.memset(ones1, 1.0)

            # v with a 65th all-ones column per head (softmax denominators)
            v_sb = constp.tile([128, KT, HPC * 65], BF16)
            v_sb_h = v_sb.rearrange("p k (h y) -> p k h y", y=65)
            nc.vector.memset(v_sb_h[:, :, :, 64], 1.0)

            # q'/k' in (d, t) layout; j-tiles: q pairs 0-3, k pairs 4-7
            qk_sb = constp.tile([128, 8, T], BF16)

            # ---- projection + rope (emitted per pair) ----
            def proj_pair(p_):
                for tcx in range(TC_):
                    tsl = slice(tcx * 512, (tcx + 1) * 512)
                    for joff, aoff, boff, dst in (
                        (p_, 0, 0, p_),          # q
                        (4 + p_, 2, 8, 4 + p_),  # k
                    ):
                        ps_m = proj_ps.tile([128, 512], F32, tag="pp")
                        for co in range(CO):
                            nc.tensor.matmul(
                                ps_m,
                                w_sb[:, co, joff * 128:(joff + 1) * 128],
                                x_sb[:, co, tsl],
                                start=(co == 0), stop=(co == CO - 1))
                        shuf = workp.tile([128, 512], F32, tag="shuf")
                        nc.vector.stream_shuffle(shuf, ps_m, SWAP_MASK)
                        s1 = workp.tile([128, 512], F32, tag="s1")
                        nc.vector.scalar_tensor_tensor(
                            s1, ps_m, bias_sb[:, boff + p_:boff + p_ + 1],
                            ab_sb[:, aoff, tsl], ALU.add, ALU.mult)
                        s2 = workp.tile([128, 512], F32, tag="s2")
                        nc.vector.scalar_tensor_tensor(
                            s2, shuf, bias_sb[:, boff + 4 + p_:boff + 5 + p_],
                            ab_sb[:, aoff + 1, tsl], ALU.add, ALU.mult)
                        nc.gpsimd.tensor_add(qk_sb[:, dst, tsl], s1, s2)

            # ---- v projection (per t-tile; interleavable) ----
            def v_proj(tt):
                psv = proj_ps.tile([128, 512], F32, tag="pp", name="psv")
                for co in range(CO):
                    nc.tensor.matmul(
                        psv,
                        x_sb[:, co, tt * 128:(tt + 1) * 128],
                        w_sb[:, co, 1024:1536],
                        start=(co == 0), stop=False)
                # bias via K=1 matmul: psv[t, j] += 1 * bv[j]
                nc.tensor.matmul(psv, ones1, bv_sb, start=False, stop=True)
                nc.vector.tensor_copy(
                    v_sb_h[:, tt, :, :64],
                    psv.rearrange("p (h d) -> p h d", d=64))

            # Deferred finalize: o_sb copies happen right after the pv
            # accumulation (freeing the psum bank); the transpose/divide/out
            # chain is emitted a few kt-groups into the NEXT iteration so the
            # psum tag rotation never stalls the next scores matmuls.
            pending = []

            def flush_pending():
                while pending:
                    o_sb, qsl_f, h = pending.pop(0)
                    tr = score_ps.tile([128, 4, 66], BF16, tag="sc", name="tr")
                    for b4 in range(4):
                        nc.tensor.transpose(
                            tr[:, b4, :65],
                            o_sb[:, b4 * 128:(b4 + 1) * 128],
                            ident[:65, :65])
                    ob = finp.tile([128, 4, 64], F32, tag="ob")
                    for b4 in range(4):
                        rc = finp.tile([128, 1], F32, tag="rc")
                        nc.vector.reciprocal(rc, tr[:, b4, 64:65])
                        nc.vector.tensor_scalar_mul(
                            ob[:, b4], tr[:, b4, :64], rc)
                    nc.sync.dma_start(
                        out.ap()[qsl_f, h * 64:(h + 1) * 64].rearrange(
                            "(b q) d -> q b d", q=128),
                        ob)

            # ---- attention (pair-outer; proj/v of later pairs overlap) ----
            proj_pair(0)
            for tt in range(4):
                v_proj(tt)
            em_r = em.ap()
            for p_ in range(PAIRS):
                for qcx in range(TC_):
                    qsl = slice(qcx * 512, (qcx + 1) * 512)
                    em_t = workp.tile([128, KT, 512], BF16, tag="em")
                    ech = max(1, KT // 4)  # kt-tiles per DMA chunk
                    for ec in range(KT // ech):
                        nc.sync.dma_start(
                            em_t[:, ec * ech:(ec + 1) * ech],
                            em_r[ec * ech * 128:(ec + 1) * ech * 128,
                                 qsl].rearrange("(kt p) q -> p kt q", p=128))
                    pvs = [pv_ps.tile([65, 512], F32, tag=f"pv{hh}",
                                      name=f"pv{hh}") for hh in range(2)]
                    for kt in range(KT):
                        sc = score_ps.tile([128, 2, 512], F32, tag="sc")
                        for hh in range(2):
                            pb = hh * 64
                            nc.tensor.matmul(
                                sc[:, hh],
                                qk_sb[pb:pb + 64, 4 + p_,
                                      kt * 128:(kt + 1) * 128],
                                qk_sb[pb:pb + 64, p_, qsl],
                                start=True, stop=True,
                                tile_position=(pb, 0))
                        e_t = ebp.tile([128, 2, 512], BF16, tag="e")
                        nc.scalar.activation(e_t, sc, AF.Exp)
                        ep_t = ebp.tile([128, 2, 512], BF16, tag="ep")
                        emb = em_t[:, kt:kt + 1, :].to_broadcast((128, 2, 512))
                        nc.vector.tensor_mul(ep_t, e_t, emb)
                        if p_ == 0 and qcx == 0 and 4 <= kt:
                            v_proj(kt)  # stream the rest of v in
                        if kt == 4:
                            flush_pending()
                        for hh in range(2):
                            h = 2 * p_ + hh
                            nc.tensor.matmul(
                                pvs[hh],
                                v_sb_h[:, kt, h, :],
                                ep_t[:, hh],
                                start=(kt == 0),
                                stop=(kt == KT - 1))
                    for hh in range(2):
                        o_sb = finp.tile([65, 512], BF16, tag="osb")
                        nc.vector.tensor_copy(o_sb, pvs[hh])
                        pending.append((o_sb, qsl, 2 * p_ + hh))
                    if qcx == 0 and p_ + 1 < PAIRS:
                        proj_pair(p_ + 1)
            flush_pending()
    nc.compile()
    return nc


def _host_prep(inputs, T=T_FULL):
    bf = ml_dtypes.bfloat16
    x = np.asarray(inputs["x"], np.float32)
    pe_cos = np.asarray(inputs["pe_cos"], np.float32)[0, 0]      # (T, D)
    pe_sin = np.asarray(inputs["pe_sin"], np.float32)[0, 0]
    pe_scale = np.asarray(inputs["pe_scale"], np.float32)[0, 0]
    mask = np.asarray(inputs["mask"], np.float32)[0]             # (B, T, T)
    w = np.asarray(inputs["w_qkv"], np.float32)                  # (3C, C)
    b = np.asarray(inputs["b_qkv"], np.float32)

    cosT, sinT, scT = pe_cos.T, pe_sin.T, pe_scale.T             # (D, T)
    # sign pattern folded into the B tables: rot(u)[d] = sgn[d]*u[d^1]
    sgn = np.tile(np.array([-1.0, 1.0], np.float32), D // 2)[:, None]
    ab_host = np.stack([
        np.tile(cosT * scT * SCALE, (2, 1)),
        np.tile(sinT * scT * SCALE * sgn, (2, 1)),
        np.tile(cosT / scT, (2, 1)),
        np.tile(sinT / scT * sgn, (2, 1)),
    ], axis=1).astype(bf)                                        # (128, 4, T)

    def swap_pairs(v):
        return np.ascontiguousarray(v.reshape(-1, 2)[:, ::-1]).reshape(v.shape)

    in_maps = []
    for c in range(N_CORES):
        bidx, g = divmod(c, 2)
        gs = slice(g * 512, (g + 1) * 512)
        wq, wk, wv = w[:C][gs], w[C:2 * C][gs], w[2 * C:][gs]
        bq, bk, bv_ = b[:C][gs], b[C:2 * C][gs], b[2 * C:][gs]
        wT_host = np.ascontiguousarray(
            np.concatenate([wq, wk, wv], 0).T).astype(bf)
        # bias cols: 0-3 q, 4-7 swap(bq), 8-11 k, 12-15 swap(bk)
        bias16 = np.zeros((128, 16), np.float32)
        bqs, bks = swap_pairs(bq), swap_pairs(bk)
        for p_ in range(PAIRS):
            ps = slice(p_ * 128, (p_ + 1) * 128)
            bias16[:, p_] = bq[ps]
            bias16[:, 4 + p_] = bqs[ps]
            bias16[:, 8 + p_] = bk[ps]
            bias16[:, 12 + p_] = bks[ps]
        in_maps.append({
            "xT": np.ascontiguousarray(x[bidx].T).astype(bf),
            "wT": wT_host,
            "ab": ab_host,
            "bias": bias16,
            "bv": bv_.astype(bf)[None, :],
            "em": np.exp(mask[bidx].T).astype(bf),
        })
    return in_maps


def kernel(**inputs):
    in_maps = _host_prep(inputs)
    nc = _build()
    res = run_bass_kernel_spmd(nc, in_maps, core_ids=list(range(N_CORES)))
    full = np.empty((B, T_FULL, C), np.float32)
    for c in range(N_CORES):
        bidx, g = divmod(c, 2)
        full[bidx, :, g * 512:(g + 1) * 512] = res.results[c]["out"]
    return full
